# revision 1
# baseline (speedup 1.0000x reference)
"""Delta-modulator scan kernel for Trainium2 (Bass/Tile).

Problem: x [128, 1024, 252] f32. Per (b, r): sequential scan over the first
232 columns with state (dc, delta, trig/quiet run counters); outputs
UP[232] | DN[232] | x[:, :, 232:252]  ->  out [128, 1024, 484] f32.

Sharding: pure data parallel over batch (16 batches / core, 8 cores).
Per-core layout: 16384 instances = [128 partitions x 128 free]; the scan
runs as 232 vectorized steps over [128, 128] state tiles.

State encoding:
  dc    : last accepted sample (f32)
  dl    : delta in {0.02, 0.1} exactly
  cc    : signed run counter (c>0: c consecutive trigs; c<0: -c consecutive quiets)
Update per step t (exact wrt reference):
  y    = x_t - dc
  up   = y > dl                      -> output
  dn   = (-y) > dl                   -> output
  trig = up + dn
  dc   = trig ? x_t : dc             (copy_predicated)
  cp   = max(cc, 0) + 1
  cc   = min(cc, 0) - 1
  cc   = trig ? cp : cc              (copy_predicated)
  A    = (cc <= -3) * 0.1
  u    = max(A, dl)
  cap  = max((cc < 3), 0.02)         ((cc<3) in {0,1}; 1.0 acts as +inf vs delta)
  dl   = min(u, cap)
"""

import os
from contextlib import ExitStack

import numpy as np

import concourse.bass as bass
import concourse.tile as tile
from concourse import bacc, mybir
from concourse.bass_utils import run_bass_kernel_spmd
import concourse.dve_ops as dve_ops_mod
from concourse.dve_spec import (
    Spec, Src0, Src1, C0, C1, C2, Zero, One, maxx, minn, select, lower,
)
from concourse.dve_spec import _has_src1
from concourse.dve_uop import DveOpSpec

AluOp = mybir.AluOpType
F32 = mybir.dt.float32


def _register_op(name: str, spec: Spec) -> "dve_ops_mod.DveOp":
    """Register a custom DVE op at runtime (compute + pin its uop sha)."""
    for existing in dve_ops_mod.OPS:
        if existing.name == name:
            return existing
    opcode = dve_ops_mod._CUSTOM_DVE_ROW_BASE + len(dve_ops_mod.OPS)
    assert opcode < 0x20
    shas = {}
    for ver in ("v3",):
        tmp = DveOpSpec(
            name=name, opcode=opcode, uops=lower(spec, ver=ver), rd1_en=_has_src1(spec)
        )
        shas[ver] = tmp.sha(ver)
    op = dve_ops_mod.DveOp(name, spec, subdim=False, uops_sha=shas)
    dve_ops_mod.OPS.append(op)
    dve_ops_mod._SUB_OPCODE_FOR_NAME[name] = opcode
    dve_ops_mod.CUSTOM_DVE_SPECS[name] = spec
    return op


# cc' = trig ? max(cc,0)+1 : min(cc,0)-1   (in0=cc, in1=trig in {0.0,1.0})
DM_COUNTER = _register_op(
    "DM_COUNTER_ANT",
    Spec(
        body=select(Src1, maxx(Src0, Zero) + One, minn(Src0, Zero) - One),
        reference=lambda in0, in1, s0, s1, imm2: np.where(
            in1 != 0.0, np.maximum(in0, 0) + 1, np.minimum(in0, 0) - 1
        ).astype(np.float32),
    ),
)

# trig = |y| > dl   (in0=y, in1=dl)
DM_TRIG = _register_op(
    "DM_TRIG_ANT",
    Spec(
        body=maxx(Src0, Zero - Src0) > Src1,
        reference=lambda in0, in1, s0, s1, imm2: (
            np.abs(in0) > in1
        ).astype(np.float32),
    ),
)

# dl' = min(max(dl, (cc<=-3)*0.1), max((cc<3), 0.02))  (in0=cc, in1=dl,
# s0=-3.0, s1=0.1, imm2=0.02)
DM_DELTA = _register_op(
    "DM_DELTA_ANT",
    Spec(
        body=minn(
            maxx(Src1, (Src0 <= C0) * C1),
            maxx(Src0 < (Zero - C0), C2),
        ),
        reference=lambda in0, in1, s0, s1, imm2: np.minimum(
            np.maximum(in1, (in0 <= s0).astype(np.float32) * s1),
            np.maximum((in0 < -s0).astype(np.float32), imm2),
        ).astype(np.float32),
    ),
)

# v = (y > dl) - (y < -dl)  in {-1, 0, +1} (never -0.0): +1 = up-trigger,
# -1 = down-trigger, 0 = no trigger. Doubles as the predication mask
# (bit pattern nonzero iff trigger).  (in0=y, in1=dl)
DM_V = _register_op(
    "DM_V2_ANT",
    Spec(
        body=(Src0 > Src1) - (Src0 < (Zero - Src1)),
        reference=lambda in0, in1, s0, s1, imm2: (
            (in0 > in1).astype(np.float32) - (in0 < -in1).astype(np.float32)
        ),
    ),
)

B, R, C = 128, 1024, 252
NSTEP = 232
NTAIL = C - NSTEP  # 20
OUTC = 2 * NSTEP + NTAIL  # 484
NCORES = 8
BPC = B // NCORES  # 16
INST = BPC * R  # 16384 instances per core
P = 128
F = INST // P  # 128

_NC_CACHE = {}


def _kernel_body(tc: "tile.TileContext", out: bass.AP, x: bass.AP) -> None:
    nc = tc.nc
    x3 = x.rearrange("(p f) c -> p f c", p=P)  # [128, 128, 252]
    o3 = out.rearrange("(p f) c -> p f c", p=P)  # [128, 128, 484]

    PASSA = 128  # pass A covers cols [0, 128); pass B covers [128, 232)
    with ExitStack() as ctx:
        state = ctx.enter_context(tc.tile_pool(name="state", bufs=1))
        xpool = ctx.enter_context(tc.tile_pool(name="xp", bufs=1))
        opool = ctx.enter_context(tc.tile_pool(name="op", bufs=1))
        tmp = ctx.enter_context(tc.tile_pool(name="tmp", bufs=6))

        dc = state.tile([P, F], F32, tag="dc")
        dl = state.tile([P, F], F32, tag="dl0")
        cc = state.tile([P, F], F32, tag="cc0")
        nc.vector.memset(dc[:], 0.0)
        nc.vector.memset(dl[:], 0.1)
        nc.vector.memset(cc[:], 0.0)
        tg = 0

        Sign = mybir.ActivationFunctionType.Sign
        Relu = mybir.ActivationFunctionType.Relu

        def step(xs, up, dn):
            nonlocal dc, dl, cc, tg
            y = tmp.tile([P, F], F32, tag="y")
            nc.gpsimd.tensor_tensor(y[:], xs, dc[:], AluOp.subtract)
            v = tmp.tile([P, F], F32, tag="v")
            nc.vector._custom_dve(DM_V, out=v[:], in0=y[:], in1=dl[:])
            nc.vector.copy_predicated(dc[:], v[:].bitcast(mybir.dt.int32), xs)
            cc2 = state.tile([P, F], F32, tag=f"cc{(tg + 1) % 2}")
            nc.vector._custom_dve(DM_COUNTER, out=cc2[:], in0=cc[:], in1=v[:])
            dl2 = state.tile([P, F], F32, tag=f"dl{(tg + 1) % 2}")
            nc.vector._custom_dve(
                DM_DELTA, out=dl2[:], in0=cc2[:], in1=dl[:],
                s0=-3.0, s1=0.1, imm2=0.02,
            )
            nc.scalar.activation(up, v[:], Relu, 0.0, 1.0)
            nc.scalar.activation(dn, v[:], Relu, 0.0, -1.0)
            cc, dl = cc2, dl2
            tg += 1

        # ---- pass A: cols [0, PASSA) ----
        xt = xpool.tile([P, F, PASSA], F32, tag="xt")
        for k0, kn in ((0, 8), (8, 8), (16, 16), (32, 32), (64, 32), (96, 32)):
            nc.sync.dma_start(xt[:, :, k0 : k0 + kn], x3[:, :, k0 : k0 + kn])
        upt = opool.tile([P, F, PASSA], F32, tag="upt")
        dnt = opool.tile([P, F, PASSA], F32, tag="dnt")

        NB = NSTEP - PASSA  # 104
        B0 = C - PASSA  # 124
        OFF = PASSA - B0  # 4
        IN_CH = 32
        # pass-B tiles share slots with pass-A tiles (same tag, bufs=1);
        # loads are emitted inside pass A's loop so the SP queue reaches
        # them early — Tile's range-level WAR deps keep it correct.
        xt2 = xpool.tile([P, F, PASSA], F32, tag="xt")
        upt2 = opool.tile([P, F, NB], F32, tag="upt")
        dnt2 = opool.tile([P, F, NB], F32, tag="dnt")

        QD = 32
        for t in range(PASSA):
            step(xt[:, :, t], upt[:, :, t], dnt[:, :, t])
            if t % QD == 15 and t > QD:
                q0 = (t // QD - 1) * QD
                nc.sync.dma_start(
                    o3[:, :, q0 : q0 + QD], upt[:, :, q0 : q0 + QD]
                )
                nc.sync.dma_start(
                    o3[:, :, NSTEP + q0 : NSTEP + q0 + QD],
                    dnt[:, :, q0 : q0 + QD],
                )
        q0 = PASSA - QD
        nc.sync.dma_start(xt2[:, :, 0:IN_CH], x3[:, :, B0 : B0 + IN_CH])
        nc.sync.dma_start(o3[:, :, q0:PASSA], upt[:, :, q0:PASSA])
        nc.sync.dma_start(
            o3[:, :, NSTEP + q0 : NSTEP + PASSA], dnt[:, :, q0:PASSA]
        )
        for k in range(IN_CH, PASSA, IN_CH):
            nc.sync.dma_start(
                xt2[:, :, k : k + IN_CH], x3[:, :, B0 + k : B0 + k + IN_CH]
            )

        # ---- pass B: cols [PASSA, NSTEP) ----
        for t in range(NB):
            step(xt2[:, :, t + OFF], upt2[:, :, t], dnt2[:, :, t])
            if t % QD == 15 and QD < t < 3 * QD:
                q0 = (t // QD - 1) * QD
                nc.sync.dma_start(
                    o3[:, :, PASSA + q0 : PASSA + q0 + QD],
                    upt2[:, :, q0 : q0 + QD],
                )
                nc.sync.dma_start(
                    o3[:, :, NSTEP + PASSA + q0 : NSTEP + PASSA + q0 + QD],
                    dnt2[:, :, q0 : q0 + QD],
                )
            if t in (80, 96):
                # trailing drains in 16-col pieces as soon as they complete
                q0 = t - 16
                nc.sync.dma_start(
                    o3[:, :, PASSA + q0 : PASSA + t], upt2[:, :, q0:t]
                )
                nc.sync.dma_start(
                    o3[:, :, NSTEP + PASSA + q0 : NSTEP + PASSA + t],
                    dnt2[:, :, q0:t],
                )
        nc.sync.dma_start(o3[:, :, PASSA + 96 : NSTEP], upt2[:, :, 96:NB])
        nc.sync.dma_start(
            o3[:, :, NSTEP + PASSA + 96 : 2 * NSTEP], dnt2[:, :, 96:NB]
        )
        # tail passthrough from the pass-B input tile (cols [232, 252))
        nc.sync.dma_start(
            o3[:, :, 2 * NSTEP : OUTC], xt2[:, :, NSTEP - B0 : PASSA]
        )


def _build_nc() -> bass.Bass:
    key = "nc"
    if key in _NC_CACHE:
        return _NC_CACHE[key]
    nc = bacc.Bacc("TRN2", target_bir_lowering=False, debug=False)
    x = nc.dram_tensor("x", [INST, C], F32, kind="ExternalInput").ap()
    out = nc.dram_tensor("out", [INST, OUTC], F32, kind="ExternalOutput").ap()
    with tile.TileContext(nc) as tc:
        _kernel_body(tc, out, x)
    nc.compile()
    _NC_CACHE[key] = nc
    return nc


def kernel(x: np.ndarray) -> np.ndarray:
    x = np.ascontiguousarray(np.asarray(x), dtype=np.float32)
    assert x.shape == (B, R, C), x.shape
    nc = _build_nc()
    in_maps = [
        {"x": np.ascontiguousarray(x[c * BPC : (c + 1) * BPC].reshape(INST, C))}
        for c in range(NCORES)
    ]
    res = run_bass_kernel_spmd(
        nc,
        in_maps,
        core_ids=list(range(NCORES)),
        trace=bool(int(os.environ.get("KERNEL_TRACE", "0"))),
    )
    global LAST_RESULTS
    LAST_RESULTS = res
    outs = [r["out"].reshape(BPC, R, OUTC) for r in res.results]
    return np.concatenate(outs, axis=0)


LAST_RESULTS = None


if __name__ == "__main__":
    xs = np.random.default_rng(0).standard_normal((B, R, C), dtype=np.float32)
    o = kernel(xs)
    print(o.shape, o.dtype)



# revision 2
# speedup vs baseline: 1.2075x; 1.2075x over previous
"""Delta-modulator scan kernel for Trainium2 — V6: raw bass (no Tile).

Per (b, r): sequential scan over the first 232 columns of x[.,.,252] with
state (dc, delta, signed run-counter); outputs UP[232] | DN[232] | x[232:252]
-> out [., ., 484] f32. Data-parallel over batch: 16 batches/core, 8 cores;
per-core 16384 instances laid out as [128 partitions x 128 free].

Structure:
- All scan ops on the DVE, back-to-back (in-order engine, no semaphores
  inside the loop): y = x - dc; v = (y>dl)-(y<-dl) -> xv slot t (in-place
  over the consumed x column); copy_predicated dc; counter; delta.
- Input loads: [0:32) (small, so the scan starts early), [32:160) and
  [104:232) as 128-column transfers (512B contiguous runs = full DMA rate).
- up/dn extracted from the v history by the Activation engine in 32-column
  chunks (relu(v), relu(-v)) into small staging tiles; SP stores each chunk.
- Tail passthrough out[464:484) = x[232:252) as a direct DRAM->DRAM DMA.
- Manual semaphores: dma_sem (+16/DMA, FIFO), dve_sem (+1 per scan step via
  copy_predicated, +1 by the init memset), act_sem (+1 per extraction).
"""

import os
from contextlib import ExitStack

import numpy as np

import concourse.bass as bass
from concourse import bacc, mybir
from concourse.bass_utils import run_bass_kernel_spmd
import concourse.dve_ops as dve_ops_mod
from concourse.dve_spec import (
    Spec, Src0, Src1, C0, C1, C2, Zero, One, maxx, minn, select, lower,
)
from concourse.dve_spec import _has_src1
from concourse.dve_uop import DveOpSpec

AluOp = mybir.AluOpType
F32 = mybir.dt.float32


def _register_op(name: str, spec: Spec) -> "dve_ops_mod.DveOp":
    """Register a custom DVE op at runtime (compute + pin its uop sha)."""
    for existing in dve_ops_mod.OPS:
        if existing.name == name:
            return existing
    opcode = dve_ops_mod._CUSTOM_DVE_ROW_BASE + len(dve_ops_mod.OPS)
    assert opcode < 0x20
    shas = {}
    for ver in ("v3",):
        tmp = DveOpSpec(
            name=name, opcode=opcode, uops=lower(spec, ver=ver), rd1_en=_has_src1(spec)
        )
        shas[ver] = tmp.sha(ver)
    op = dve_ops_mod.DveOp(name, spec, subdim=False, uops_sha=shas)
    dve_ops_mod.OPS.append(op)
    dve_ops_mod._SUB_OPCODE_FOR_NAME[name] = opcode
    dve_ops_mod.CUSTOM_DVE_SPECS[name] = spec
    return op


# cc' = trig ? max(cc,0)+1 : min(cc,0)-1   (in0=cc, in1=v; trig = in1 != 0)
DM_COUNTER = _register_op(
    "DM_COUNTER_ANT",
    Spec(
        body=select(Src1, maxx(Src0, Zero) + One, minn(Src0, Zero) - One),
        reference=lambda in0, in1, s0, s1, imm2: np.where(
            in1 != 0.0, np.maximum(in0, 0) + 1, np.minimum(in0, 0) - 1
        ).astype(np.float32),
    ),
)

# dl' = min(max(dl, (cc<=-3)*0.1), max((cc<3), 0.02))  (in0=cc, in1=dl,
# s0=-3.0, s1=0.1, imm2=0.02)
DM_DELTA = _register_op(
    "DM_DELTA_ANT",
    Spec(
        body=minn(
            maxx(Src1, (Src0 <= C0) * C1),
            maxx(Src0 < (Zero - C0), C2),
        ),
        reference=lambda in0, in1, s0, s1, imm2: np.minimum(
            np.maximum(in1, (in0 <= s0).astype(np.float32) * s1),
            np.maximum((in0 < -s0).astype(np.float32), imm2),
        ).astype(np.float32),
    ),
)

# v = (y > dl) - (y < -dl) in {-1, 0, +1} (never -0.0). Doubles as the
# predication mask (bit pattern nonzero iff trigger).  (in0=y, in1=dl)
DM_V = _register_op(
    "DM_V2_ANT",
    Spec(
        body=(Src0 > Src1) - (Src0 < (Zero - Src1)),
        reference=lambda in0, in1, s0, s1, imm2: (
            (in0 > in1).astype(np.float32) - (in0 < -in1).astype(np.float32)
        ),
    ),
)

B, R, C = 128, 1024, 252
NSTEP = 232
NTAIL = C - NSTEP  # 20
OUTC = 2 * NSTEP + NTAIL  # 484
NCORES = 8
BPC = B // NCORES  # 16
INST = BPC * R  # 16384 instances per core
P = 128
F = INST // P  # 128

# store chunking: 7x32 + 1x8 columns per plane
CHUNKS = [(0, 32), (32, 32), (64, 32), (96, 32), (128, 32), (160, 32),
          (192, 32), (224, 8)]

_NC_CACHE = {}


def _build_nc() -> bass.Bass:
    key = "nc"
    if key in _NC_CACHE:
        return _NC_CACHE[key]
    nc = bacc.Bacc("TRN2", target_bir_lowering=False, debug=False)
    x = nc.dram_tensor("x", [INST, C], F32, kind="ExternalInput").ap()
    out = nc.dram_tensor("out", [INST, OUTC], F32, kind="ExternalOutput").ap()
    x3 = x.rearrange("(p f) c -> p f c", p=P)  # [128, 128, 252]
    o3 = out.rearrange("(p f) c -> p f c", p=P)  # [128, 128, 484]

    Relu = mybir.ActivationFunctionType.Relu
    NDMA = 4 + 2 * len(CHUNKS)

    with ExitStack() as ctx:
        # xv slot k holds x_{k-1}; v_t overwrites slot t (x_{t-1} dead).
        xv_t = ctx.enter_context(nc.sbuf_tensor("xv", [P, F, NSTEP + 1], F32))
        s_up_t = ctx.enter_context(nc.sbuf_tensor("s_up", [P, F, 32], F32))
        s_dn_t = ctx.enter_context(nc.sbuf_tensor("s_dn", [P, F, 32], F32))
        dc_t = ctx.enter_context(nc.sbuf_tensor("dc", [P, F], F32))
        dl_ts = [
            ctx.enter_context(nc.sbuf_tensor(f"dl{i}", [P, F], F32))
            for i in range(2)
        ]
        cc_ts = [
            ctx.enter_context(nc.sbuf_tensor(f"cc{i}", [P, F], F32))
            for i in range(2)
        ]
        y_ts = [
            ctx.enter_context(nc.sbuf_tensor(f"y{i}", [P, F], F32))
            for i in range(2)
        ]
        dma_sem = ctx.enter_context(nc.semaphore("dma_sem"))
        dve_sem = ctx.enter_context(nc.semaphore("dve_sem"))
        act_sem = ctx.enter_context(nc.semaphore("act_sem"))

        xv = xv_t.ap()
        s_up = s_up_t.ap()
        s_dn = s_dn_t.ap()
        dc = dc_t.ap()
        dls = [t.ap() for t in dl_ts]
        ccs = [t.ap() for t in cc_ts]
        ys = [t.ap() for t in y_ts]

        with nc.Block() as block:

            @block.sync
            def _(sync):
                # loads: x_k -> slot k+1
                sync.dma_start(xv[:, :, 1:33], x3[:, :, 0:32]).then_inc(
                    dma_sem, 16
                )
                sync.dma_start(xv[:, :, 33:161], x3[:, :, 32:160]).then_inc(
                    dma_sem, 16
                )
                sync.dma_start(xv[:, :, 105:233], x3[:, :, 104:232]).then_inc(
                    dma_sem, 16
                )
                # tail passthrough DRAM->DRAM
                sync.dma_start(
                    o3[:, :, 2 * NSTEP : OUTC], x3[:, :, NSTEP:C]
                ).then_inc(dma_sem, 16)
                for k, (c0, cn) in enumerate(CHUNKS):
                    sync.wait_ge(act_sem, 2 * k + 1)
                    sync.dma_start(
                        o3[:, :, c0 : c0 + cn], s_up[:, :, 0:cn]
                    ).then_inc(dma_sem, 16)
                    sync.wait_ge(act_sem, 2 * k + 2)
                    sync.dma_start(
                        o3[:, :, NSTEP + c0 : NSTEP + c0 + cn], s_dn[:, :, 0:cn]
                    ).then_inc(dma_sem, 16)
                sync.wait_ge(dma_sem, 16 * NDMA)

            @block.vector
            def _(vector):
                vector.memset(dc[:], 0.0)
                vector.memset(dls[0][:], 0.1)
                vector.memset(ccs[0][:], 0.0).then_inc(dve_sem)
                dli = cci = 0
                for t in range(NSTEP):
                    if t == 0:
                        vector.wait_ge(dma_sem, 16)
                    elif t == 32:
                        vector.wait_ge(dma_sem, 32)
                    elif t == 105:
                        vector.wait_ge(dma_sem, 48)
                    xs = xv[:, :, t + 1]
                    y = ys[t % 2]
                    dl, cc = dls[dli], ccs[cci]
                    dl2, cc2 = dls[1 - dli], ccs[1 - cci]
                    vslot = xv[:, :, t]
                    vector.tensor_tensor(y[:], xs, dc[:], AluOp.subtract)
                    vector._custom_dve(DM_V, out=vslot, in0=y[:], in1=dl[:])
                    vector.copy_predicated(
                        dc[:], vslot.bitcast(mybir.dt.int32), xs
                    ).then_inc(dve_sem)
                    vector._custom_dve(
                        DM_COUNTER, out=cc2[:], in0=cc[:], in1=vslot
                    )
                    vector._custom_dve(
                        DM_DELTA, out=dl2[:], in0=cc2[:], in1=dl[:],
                        s0=-3.0, s1=0.1, imm2=0.02,
                    )
                    dli, cci = 1 - dli, 1 - cci

            @block.scalar
            def _(scalar):
                for k, (c0, cn) in enumerate(CHUNKS):
                    # v history final through col c0+cn-1 once that step's
                    # copy_predicated (after DM_V) completed: dve = 1+(c0+cn)
                    scalar.wait_ge(dve_sem, 1 + c0 + cn)
                    if k >= 1:
                        # stage WAR: previous up-store (DMA #(2k+3)) done
                        scalar.wait_ge(dma_sem, 16 * (2 * k + 3))
                    scalar.activation(
                        s_up[:, :, 0:cn], xv[:, :, c0 : c0 + cn], Relu, 0.0, 1.0
                    ).then_inc(act_sem)
                    if k >= 1:
                        scalar.wait_ge(dma_sem, 16 * (2 * k + 4))
                    scalar.activation(
                        s_dn[:, :, 0:cn], xv[:, :, c0 : c0 + cn], Relu, 0.0,
                        -1.0,
                    ).then_inc(act_sem)

    nc.compile()
    _NC_CACHE[key] = nc
    return nc


def kernel(x: np.ndarray) -> np.ndarray:
    x = np.ascontiguousarray(np.asarray(x), dtype=np.float32)
    assert x.shape == (B, R, C), x.shape
    nc = _build_nc()
    in_maps = [
        {"x": np.ascontiguousarray(x[c * BPC : (c + 1) * BPC].reshape(INST, C))}
        for c in range(NCORES)
    ]
    res = run_bass_kernel_spmd(
        nc,
        in_maps,
        core_ids=list(range(NCORES)),
        trace=bool(int(os.environ.get("KERNEL_TRACE", "0"))),
    )
    global LAST_RESULTS
    LAST_RESULTS = res
    outs = [r["out"].reshape(BPC, R, OUTC) for r in res.results]
    return np.concatenate(outs, axis=0)


LAST_RESULTS = None


if __name__ == "__main__":
    xs = np.random.default_rng(0).standard_normal((B, R, C), dtype=np.float32)
    o = kernel(xs)
    print(o.shape, o.dtype)


# revision 3
# speedup vs baseline: 1.3068x; 1.0822x over previous
"""Delta-modulator scan kernel for Trainium2 — V11: raw bass (no Tile).

Per (b, r): sequential scan over the first 232 columns of x[.,.,252] with
state (dc, delta, signed run-counter); outputs UP[232] | DN[232] | x[232:252]
-> out [., ., 484] f32. Data-parallel over batch: 16 batches/core, 8 cores;
per-core 16384 instances laid out as [128 partitions x 128 free].

Structure:
- All scan ops on the DVE, back-to-back (in-order engine, no semaphores
  inside the loop): y = x - dc; v = (y>dl)-(y<-dl) -> xv slot t (in-place
  over the consumed x column); copy_predicated dc; counter; delta.
- Input loads: [0:32) (small, so the scan starts early), [32:160) and
  [104:232) as 128-column transfers (512B contiguous runs = full DMA rate).
- up/dn extracted from the v history by the Activation engine in 32-column
  chunks (relu(v), relu(-v)) into small staging tiles; SP stores each chunk.
- Tail passthrough out[464:484) = x[232:252) as a direct DRAM->DRAM DMA.
- Manual semaphores: dma_sem (+16/DMA, FIFO), dve_sem (+1 per scan step via
  copy_predicated, +1 by the init memset), act_sem (+1 per extraction).
"""

import os
from contextlib import ExitStack

import numpy as np

import concourse.bass as bass
from concourse import bacc, mybir
from concourse.bass_utils import run_bass_kernel_spmd
import concourse.dve_ops as dve_ops_mod
from concourse.dve_spec import (
    Spec, Src0, Src1, C0, C1, C2, Zero, One, maxx, minn, select, lower,
)
from concourse.dve_spec import _has_src1
from concourse.dve_uop import DveOpSpec

AluOp = mybir.AluOpType
F32 = mybir.dt.float32


def _register_op(name: str, spec: Spec) -> "dve_ops_mod.DveOp":
    """Register a custom DVE op at runtime (compute + pin its uop sha)."""
    for existing in dve_ops_mod.OPS:
        if existing.name == name:
            return existing
    opcode = dve_ops_mod._CUSTOM_DVE_ROW_BASE + len(dve_ops_mod.OPS)
    assert opcode < 0x20
    shas = {}
    for ver in ("v3",):
        tmp = DveOpSpec(
            name=name, opcode=opcode, uops=lower(spec, ver=ver), rd1_en=_has_src1(spec)
        )
        shas[ver] = tmp.sha(ver)
    op = dve_ops_mod.DveOp(name, spec, subdim=False, uops_sha=shas)
    dve_ops_mod.OPS.append(op)
    dve_ops_mod._SUB_OPCODE_FOR_NAME[name] = opcode
    dve_ops_mod.CUSTOM_DVE_SPECS[name] = spec
    return op


# cc' = trig ? max(cc,0)+1 : min(cc,0)-1   (in0=cc, in1=v; trig = in1 != 0)
DM_COUNTER = _register_op(
    "DM_COUNTER_ANT",
    Spec(
        body=select(Src1, maxx(Src0, Zero) + One, minn(Src0, Zero) - One),
        reference=lambda in0, in1, s0, s1, imm2: np.where(
            in1 != 0.0, np.maximum(in0, 0) + 1, np.minimum(in0, 0) - 1
        ).astype(np.float32),
    ),
)

# dl' = min(max(dl, (cc<=-3)*0.1), max((cc<3), 0.02))  (in0=cc, in1=dl,
# s0=-3.0, s1=0.1, imm2=0.02)
DM_DELTA = _register_op(
    "DM_DELTA_ANT",
    Spec(
        body=minn(
            maxx(Src1, (Src0 <= C0) * C1),
            maxx(Src0 < (Zero - C0), C2),
        ),
        reference=lambda in0, in1, s0, s1, imm2: np.minimum(
            np.maximum(in1, (in0 <= s0).astype(np.float32) * s1),
            np.maximum((in0 < -s0).astype(np.float32), imm2),
        ).astype(np.float32),
    ),
)

# v = (y > dl) - (y < -dl) in {-1, 0, +1} (never -0.0). Doubles as the
# predication mask (bit pattern nonzero iff trigger).  (in0=y, in1=dl)
DM_V = _register_op(
    "DM_V2_ANT",
    Spec(
        body=(Src0 > Src1) - (Src0 < (Zero - Src1)),
        reference=lambda in0, in1, s0, s1, imm2: (
            (in0 > in1).astype(np.float32) - (in0 < -in1).astype(np.float32)
        ),
    ),
)

B, R, C = 128, 1024, 252
NSTEP = 232
NTAIL = C - NSTEP  # 20
OUTC = 2 * NSTEP + NTAIL  # 484
NCORES = 8
BPC = B // NCORES  # 16
INST = BPC * R  # 16384 instances per core
P = 128
F = INST // P  # 128

# store chunking: 32-col early, then 24/20-col late chunks. 20-col pairs
# hit the 7ns/descriptor floor at the same per-column rate as 32-col but
# carry less mass after the scan's last step, which bounds the finish.
CHUNKS = [(0, 32), (32, 32), (64, 32), (96, 32), (128, 24), (152, 24),
          (176, 20), (196, 20), (216, 16)]

_NC_CACHE = {}


def _build_nc() -> bass.Bass:
    key = "nc"
    if key in _NC_CACHE:
        return _NC_CACHE[key]
    nc = bacc.Bacc("TRN2", target_bir_lowering=False, debug=False)
    x = nc.dram_tensor("x", [INST, C], F32, kind="ExternalInput").ap()
    out = nc.dram_tensor("out", [INST, OUTC], F32, kind="ExternalOutput").ap()
    x3 = x.rearrange("(p f) c -> p f c", p=P)  # [128, 128, 252]
    o3 = out.rearrange("(p f) c -> p f c", p=P)  # [128, 128, 484]

    Relu = mybir.ActivationFunctionType.Relu
    NDMA = 4 + 2 * len(CHUNKS)

    with ExitStack() as ctx:
        # xv slot k holds x_{k-1}; v_t overwrites slot t (x_{t-1} dead).
        xv_t = ctx.enter_context(nc.sbuf_tensor("xv", [P, F, NSTEP + 1], F32))
        s_up_t = ctx.enter_context(nc.sbuf_tensor("s_up", [P, F, 32], F32))
        s_dn_t = ctx.enter_context(nc.sbuf_tensor("s_dn", [P, F, 32], F32))
        dc_t = ctx.enter_context(nc.sbuf_tensor("dc", [P, F], F32))
        dl_ts = [
            ctx.enter_context(nc.sbuf_tensor(f"dl{i}", [P, F], F32))
            for i in range(2)
        ]
        cc_ts = [
            ctx.enter_context(nc.sbuf_tensor(f"cc{i}", [P, F], F32))
            for i in range(2)
        ]
        y_ts = [
            ctx.enter_context(nc.sbuf_tensor(f"y{i}", [P, F], F32))
            for i in range(2)
        ]
        dma_sem = ctx.enter_context(nc.semaphore("dma_sem"))
        dve_sem = ctx.enter_context(nc.semaphore("dve_sem"))
        act_sem = ctx.enter_context(nc.semaphore("act_sem"))

        xv = xv_t.ap()
        s_up = s_up_t.ap()
        s_dn = s_dn_t.ap()
        dc = dc_t.ap()
        dls = [t.ap() for t in dl_ts]
        ccs = [t.ap() for t in cc_ts]
        ys = [t.ap() for t in y_ts]

        with nc.Block() as block:

            @block.sync
            def _(sync_):
                sync = sync_
                # loads: x_k -> slot k+1
                sync.dma_start(xv[:, :, 1:25], x3[:, :, 0:24]).then_inc(
                    dma_sem, 16
                )
                sync.dma_start(xv[:, :, 25:153], x3[:, :, 24:152]).then_inc(
                    dma_sem, 16
                )
                sync.dma_start(xv[:, :, 105:233], x3[:, :, 104:232]).then_inc(
                    dma_sem, 16
                )
                # tail passthrough DRAM->DRAM
                sync.dma_start(
                    o3[:, :, 2 * NSTEP : OUTC], x3[:, :, NSTEP:C]
                ).then_inc(dma_sem, 16)
                for k, (c0, cn) in enumerate(CHUNKS):
                    sync.wait_ge(act_sem, 4 * k + 2)
                    sync.dma_start(
                        o3[:, :, c0 : c0 + cn], s_up[:, :, 0:cn]
                    ).then_inc(dma_sem, 16)
                    sync.wait_ge(act_sem, 4 * k + 4)
                    sync.dma_start(
                        o3[:, :, NSTEP + c0 : NSTEP + c0 + cn], s_dn[:, :, 0:cn]
                    ).then_inc(dma_sem, 16)
                sync.wait_ge(dma_sem, 16 * NDMA)

            @block.vector
            def _(vector):
                vector.memset(dc[:], 0.0)
                vector.memset(dls[0][:], 0.1)
                vector.memset(ccs[0][:], 0.0).then_inc(dve_sem)
                dli = cci = 0
                for t in range(NSTEP):
                    if t == 0:
                        vector.wait_ge(dma_sem, 16)
                    elif t == 24:
                        vector.wait_ge(dma_sem, 32)
                    elif t == 105:
                        vector.wait_ge(dma_sem, 48)
                    xs = xv[:, :, t + 1]
                    y = ys[t % 2]
                    dl, cc = dls[dli], ccs[cci]
                    dl2, cc2 = dls[1 - dli], ccs[1 - cci]
                    vslot = xv[:, :, t]
                    vector.tensor_tensor(y[:], xs, dc[:], AluOp.subtract)
                    if t == NSTEP - 1:
                        # final step: only v is consumed (by extraction)
                        vector._custom_dve(
                            DM_V, out=vslot, in0=y[:], in1=dl[:]
                        ).then_inc(dve_sem)
                        break
                    vector._custom_dve(DM_V, out=vslot, in0=y[:], in1=dl[:])
                    vector.copy_predicated(
                        dc[:], vslot.bitcast(mybir.dt.int32), xs
                    ).then_inc(dve_sem)
                    vector._custom_dve(
                        DM_COUNTER, out=cc2[:], in0=cc[:], in1=vslot
                    )
                    vector._custom_dve(
                        DM_DELTA, out=dl2[:], in0=cc2[:], in1=dl[:],
                        s0=-3.0, s1=0.1, imm2=0.02,
                    )
                    dli, cci = 1 - dli, 1 - cci

            @block.scalar
            def _(scalar):
                # each plane's extraction is split (cn-1)+1 so the store can
                # fire as soon as the chunk's LAST column's v lands.
                for k, (c0, cn) in enumerate(CHUNKS):
                    scalar.wait_ge(dve_sem, c0 + cn)  # v through col c0+cn-2
                    if k >= 1:
                        # stage WAR: previous up-store (DMA #(2k+3)) done
                        scalar.wait_ge(dma_sem, 16 * (2 * k + 3))
                    scalar.activation(
                        s_up[:, :, 0 : cn - 1],
                        xv[:, :, c0 : c0 + cn - 1], Relu, 0.0, 1.0,
                    ).then_inc(act_sem)
                    scalar.wait_ge(dve_sem, 1 + c0 + cn)  # v(c0+cn-1) done
                    scalar.activation(
                        s_up[:, :, cn - 1 : cn],
                        xv[:, :, c0 + cn - 1 : c0 + cn], Relu, 0.0, 1.0,
                    ).then_inc(act_sem)
                    if k >= 1:
                        scalar.wait_ge(dma_sem, 16 * (2 * k + 4))
                    scalar.activation(
                        s_dn[:, :, 0 : cn - 1],
                        xv[:, :, c0 : c0 + cn - 1], Relu, 0.0, -1.0,
                    ).then_inc(act_sem)
                    scalar.activation(
                        s_dn[:, :, cn - 1 : cn],
                        xv[:, :, c0 + cn - 1 : c0 + cn], Relu, 0.0, -1.0,
                    ).then_inc(act_sem)

    nc.compile()
    _NC_CACHE[key] = nc
    return nc


def kernel(x: np.ndarray) -> np.ndarray:
    x = np.ascontiguousarray(np.asarray(x), dtype=np.float32)
    assert x.shape == (B, R, C), x.shape
    nc = _build_nc()
    in_maps = [
        {"x": np.ascontiguousarray(x[c * BPC : (c + 1) * BPC].reshape(INST, C))}
        for c in range(NCORES)
    ]
    res = run_bass_kernel_spmd(
        nc,
        in_maps,
        core_ids=list(range(NCORES)),
        trace=bool(int(os.environ.get("KERNEL_TRACE", "0"))),
    )
    global LAST_RESULTS
    LAST_RESULTS = res
    outs = [r["out"].reshape(BPC, R, OUTC) for r in res.results]
    return np.concatenate(outs, axis=0)


LAST_RESULTS = None


if __name__ == "__main__":
    xs = np.random.default_rng(0).standard_normal((B, R, C), dtype=np.float32)
    o = kernel(xs)
    print(o.shape, o.dtype)
